# revision 35
# baseline (speedup 1.0000x reference)
"""CBOW negative-sampling loss on 8 Trainium2 NeuronCores.

Reference computation:
    v      = V_emb[ctx] * mask_v                  # [B,1,E]
    u      = U_emb[tgt] * mask_u                  # [B,1,E]
    u_neg  = -(U_emb[neg] * mask_neg)             # [B,K,E]
    pos    = <u, v>
    neg    = sum_k <u_neg_k, v>
    loss   = -mean(log_sigmoid(pos) + log_sigmoid(neg))
           = mean(softplus(-pos) + softplus(negsum)),  negsum = -neg

Strategy: data-parallel over B across 8 cores.  Each batch row needs 12
row-gathers (ctx, tgt, 10 neg).  The bottleneck on TRN2 is GPSIMD Q7
descriptor generation for dma_gather (~8 ns/descriptor per core pair),
attacked two ways:

1. Halve descriptors: the 12 slots of a batch element form 6 PAIRS of
   adjacent X-tile columns; the host dedups the ordered id-pairs per
   super group (sg = 8 row-tiles, 6144 pairs) and lays both rows of
   each unique pair adjacently in a per-sg table slab of 512B pair-rows
   (bf16), so one 512B descriptor fetches a whole pair.
2. Parallelize generation: gathers go on SWDGE queues 1-3 (queue q ->
   Q7 core pair q), which dispatch in ~60ns and generate descriptors
   asynchronously on three core pairs concurrently.  Queue 0 is avoided
   (synchronous, barriers the Pool engine); the queue pattern has
   period 8 so the tile framework's 8 round-robin DMASW semaphore lanes
   stay queue-consistent.  Half-sg units (3072 descs) with an 8-deep X
   pool keep every queue fed without buffer stalls.

The gather writes X in position-order layout [p, (tile slot e)]; masks
are host-packed into the identical layout (one coalesced DMA per unit).
Compute per tile: DVE X *= M (bf16 2x mode), one broadcast-vm multiply
(stride-0 AP) over the 11 u-slots, pos dot via DVE reduce / scalar
accum (alternating), negsum via scalar accum_out; f32 softplus tail;
per-core [128,1] partials summed on host.
"""

import numpy as np

B, K, VOCAB, E = 65536, 10, 100000, 128
NCORES = 8
P = 128
SLOTS = 2 + K
PAIRS = SLOTS // 2                  # 6 pairs per batch element

SG_TILES = 8                        # row-tiles per super group
SG_PAIRS = SG_TILES * P * PAIRS     # gather descriptors per sg = 6144
TCAP = SG_PAIRS * 2 + 256           # per-sg table slab rows (2 per pair + pad)

# gather units: (sg, first tile within sg, n tiles).  ~Half-sg units
# amortize the ~1.7us per-call fixed cost and keep the 8-deep DMASW-lane
# dependency chain shallow (2 per lane); sizes are skewed so per-queue
# tile totals balance at (22, 22, 20) instead of the (24, 24, 16) that
# equal 4-tile units would force.
UNITS = [
    (0, 0, 4), (0, 4, 4), (1, 0, 5), (1, 5, 3),
    (2, 0, 3), (2, 3, 5), (3, 0, 4), (3, 4, 4),
    (4, 0, 4), (4, 4, 4), (5, 0, 5), (5, 5, 3),
    (6, 0, 3), (6, 3, 5), (7, 0, 4), (7, 4, 4),
]
# SWDGE queue per stream-residue (period 8 keeps the tile framework's
# DMASW semaphore lanes queue-consistent).  Queue 0 is avoided: its
# dispatches block the Pool engine.
QPAT = (1, 2, 3, 1, 2, 3, 1, 2)

_prog_cache = {}


def _bf16():
    import ml_dtypes

    return np.dtype(ml_dtypes.bfloat16)


def _build_program(bsh, vocab, k, e, sg_tiles, ncores):
    import concourse.bacc as bacc
    import concourse.tile as tile
    from concourse import library_config, mybir
    from concourse.ap import AP

    f32 = mybir.dt.float32
    bf16 = mybir.dt.bfloat16
    i16 = mybir.dt.int16
    slots = 2 + k
    pairs = slots // 2
    tiles = bsh // P
    assert tiles % sg_tiles == 0
    nsg = tiles // sg_tiles
    sg_pairs = sg_tiles * P * pairs
    xcols = sg_tiles * slots           # X columns per sg
    gi_cols = sg_pairs // 16           # int16 idx columns per sg (16-wrap)

    h_tiles = sg_tiles // 2                # tiles per half-sg unit
    h_pairs = sg_pairs // 2                # descriptors per half-sg = 3072
    h_cols = h_pairs // 16                 # idx columns per half-sg
    hx = h_tiles * slots * e               # X elements per half-sg

    nc = bacc.Bacc(
        "TRN2", target_bir_lowering=False, debug=False, num_devices=ncores,
        num_swdge_queues=4,
    )
    # table of PAIR-rows: one row = the two gathered table rows of a pair
    w = nc.dram_tensor("w", [nsg * TCAP // 2, 2 * e], bf16, kind="ExternalInput")
    gi = nc.dram_tensor("gi", [P, nsg * gi_cols], i16, kind="ExternalInput")
    # masks pre-packed host-side into the X layout [p, (tile slot e)]
    mk = nc.dram_tensor("mk", [P, tiles * slots * e], bf16, kind="ExternalInput")
    out = nc.dram_tensor("out", [P, 1], f32, kind="ExternalOutput")

    mult = mybir.AluOpType.mult
    add = mybir.AluOpType.add
    AF = mybir.ActivationFunctionType

    with tile.TileContext(nc) as tc:
        with (
            tc.tile_pool(name="px", bufs=7) as xpool,
            tc.tile_pool(name="pm", bufs=4) as mpool,
            tc.tile_pool(name="py", bufs=4) as ypool,
            tc.tile_pool(name="acc", bufs=1) as apool,
        ):
            nc.gpsimd.load_library(library_config.mlp)

            post = apool.tile([P, tiles], f32, tag="post")
            negt = apool.tile([P, tiles], f32, tag="negt")
            # pair-indices: first unit's slice in its own tiny DMA so the
            # first gather can start early; the rest preloaded behind it
            t_cols = P * pairs // 16           # idx columns per tile
            giall = apool.tile([P, nsg * gi_cols], i16, tag="giall")
            nc.sync.dma_start(out=giall[:, 0:t_cols], in_=gi[:, 0:t_cols])
            nc.sync.dma_start(out=giall[:, t_cols:], in_=gi[:, t_cols:])

            # touch every activation table the softplus tail needs so the
            # ~1.3us ACT_TABLE_LOADs happen now, off the critical tail
            warm = apool.tile([P, 1], f32, tag="warm")
            nc.vector.memset(warm[:], 0.0)
            for fn, kw in (
                (AF.Copy, {}), (AF.Abs, {}), (AF.Exp, {"scale": -1.0}),
                (AF.Ln, {"bias": 1.0}), (AF.Relu, {"scale": -1.0}),
            ):
                nc.scalar.activation(out=warm[:], in_=warm[:], func=fn, **kw)

            for u, (sg, tstart, nt) in enumerate(UNITS):
                # --- paired gather for this unit: desc d -> pair-row idx
                # of the sg's table slab -> X[:, d//128 pair-column] (512B).
                # Queues 1-3 dispatch asynchronously (three Q7 core pairs
                # generate descriptors concurrently), so gathers overlap.
                ux = nt * slots * e            # X elements for this unit
                u_pairs = nt * P * pairs
                Xt = xpool.tile([P, 5 * slots * e], bf16, tag="X")
                X = Xt[:, 0:ux]
                gbase = sg * gi_cols + tstart * t_cols
                nc.gpsimd.dma_gather(
                    out_ap=X.rearrange("p (c e2) -> p c e2", e2=2 * e),
                    in_ap=w[sg * (TCAP // 2) : (sg + 1) * (TCAP // 2), :],
                    idxs_ap=giall[:, gbase : gbase + nt * t_cols],
                    num_idxs=u_pairs,
                    num_idxs_reg=u_pairs,
                    elem_size=2 * e,
                    single_packet=False,
                    queue_num=QPAT[u % 8],
                )

                # --- masks (host-packed in X layout, one coalesced DMA) ---
                Mt = mpool.tile([P, 5 * slots * e], bf16, tag="M")
                M = Mt[:, 0:ux]
                t0 = sg * sg_tiles + tstart
                nc.sync.dma_start(
                    out=M,
                    in_=mk[:, t0 * slots * e : (t0 + nt) * slots * e],
                )

                # --- compute per tile ---
                # negsum_b = sum_k <X_k, vm>; pos_b = <X_tgt, vm>.
                # One masked multiply, then one broadcast-vm multiply over
                # the 11 u-slots; the scalar engine accumulates the free-dim
                # sums (pos from the tgt slot, neg from the 10 neg slots).
                xv = X
                for tl in range(nt):
                    t = t0 + tl
                    base = tl * slots * e
                    nc.vector.tensor_tensor(
                        out=xv[:, base : base + slots * e],
                        in0=xv[:, base : base + slots * e],
                        in1=M[:, base : base + slots * e],
                        op=mult,
                    )
                    vm = xv[:, base : base + e]
                    vmb = AP(
                        vm.tensor, vm.offset,
                        [list(vm.ap[0]), [0, slots - 1], list(vm.ap[1])],
                    )
                    yu = ypool.tile([P, (slots - 1) * e], bf16, tag="yu")
                    nc.vector.tensor_tensor(
                        out=yu[:].rearrange("p (s e) -> p s e", e=e),
                        in0=xv[:, base + e : base + slots * e]
                        .rearrange("p (s e) -> p s e", e=e),
                        in1=vmb,
                        op=mult,
                    )
                    # pos accumulation alternates DVE/ACT to balance engines
                    if tl % 2 == 0:
                        nc.vector.tensor_reduce(
                            out=post[:, t : t + 1], in_=yu[:, 0:e],
                            axis=mybir.AxisListType.X, op=add,
                        )
                    else:
                        nc.scalar.activation(
                            out=yu[:, 0:e], in_=yu[:, 0:e], func=AF.Copy,
                            accum_out=post[:, t : t + 1],
                        )
                    nc.scalar.activation(
                        out=yu[:, e : (slots - 1) * e],
                        in_=yu[:, e : (slots - 1) * e], func=AF.Copy,
                        accum_out=negt[:, t : t + 1],
                    )

            # --- softplus tail (f32): mean(softplus(-pos) + softplus(neg)) ---
            # softplus(z) = relu(z) + ln(1 + exp(-|z|))
            sabs = apool.tile([P, tiles], f32, tag="sabs")
            sexp = apool.tile([P, tiles], f32, tag="sexp")
            sln = apool.tile([P, tiles], f32, tag="sln")
            srel = apool.tile([P, tiles], f32, tag="srel")
            ssum = apool.tile([P, tiles], f32, tag="ssum")
            acc1 = apool.tile([P, 1], f32, tag="acc1")
            acc2 = apool.tile([P, 1], f32, tag="acc2")
            tot = apool.tile([P, 1], f32, tag="tot")

            for src, sgn, acc in ((post, -1.0, acc1), (negt, 1.0, acc2)):
                nc.scalar.activation(out=sabs[:], in_=src[:], func=AF.Abs)
                nc.scalar.activation(
                    out=sexp[:], in_=sabs[:], func=AF.Exp, scale=-1.0
                )
                nc.scalar.activation(out=sln[:], in_=sexp[:], func=AF.Ln, bias=1.0)
                nc.scalar.activation(
                    out=srel[:], in_=src[:], func=AF.Relu, scale=sgn
                )
                nc.vector.tensor_tensor(
                    out=ssum[:], in0=sln[:], in1=srel[:], op=add
                )
                nc.scalar.activation(
                    out=ssum[:], in_=ssum[:], func=AF.Copy, accum_out=acc[:]
                )
            nc.vector.tensor_tensor(out=tot[:], in0=acc1[:], in1=acc2[:], op=add)
            nc.sync.dma_start(out=out[:], in_=tot[:])

    nc.compile()
    return nc


def _get_program(bsh, vocab, k, e, sg_tiles, ncores):
    key = (bsh, vocab, k, e, sg_tiles, ncores)
    if key not in _prog_cache:
        _prog_cache[key] = _build_program(bsh, vocab, k, e, sg_tiles, ncores)
    return _prog_cache[key]


def _wrap16(vals, ncols):
    """int16 list -> [128, ncols] tile data: value i at [i%16, i//16],
    replicated across the 8 16-partition groups."""
    assert vals.shape[0] == ncols * 16
    arr = np.ascontiguousarray(vals.reshape(ncols, 16).T)
    return np.tile(arr, (8, 1))


def _host_prep(
    ctx_words, target_words, neg_words, V_emb, U_emb, mask_v, mask_u, mask_neg,
    ncores, sg_tiles,
):
    bf16 = _bf16()
    b, k = neg_words.shape
    vocab, e = V_emb.shape
    bsh = b // ncores
    slots = 2 + k
    pairs = slots // 2
    tiles = bsh // P
    nsg = tiles // sg_tiles
    sg_pairs = sg_tiles * P * pairs
    gi_cols = sg_pairs // 16

    W = np.concatenate(
        [np.asarray(V_emb, dtype=np.float32), np.asarray(U_emb, dtype=np.float32)],
        axis=0,
    ).astype(bf16)

    ctx = np.clip(np.asarray(ctx_words).reshape(b), 0, vocab - 1).astype(np.int64)
    tgt = np.clip(np.asarray(target_words).reshape(b), 0, vocab - 1).astype(np.int64)
    neg = np.clip(np.asarray(neg_words).reshape(b, k), 0, vocab - 1).astype(np.int64)

    # gather ids per position: ids[b_row, slot]
    ids_all = np.empty((b, slots), dtype=np.int64)
    ids_all[:, 0] = ctx
    ids_all[:, 1] = vocab + tgt
    ids_all[:, 2:] = vocab + neg

    # pack masks into the X layout: [p, (tile slot e)] per core
    ntiles = b // P
    mpack = np.empty((ntiles, P, slots, e), dtype=bf16)
    mpack[:, :, 0, :] = np.asarray(mask_v, dtype=np.float32).reshape(ntiles, P, e)
    mpack[:, :, 1, :] = np.asarray(mask_u, dtype=np.float32).reshape(ntiles, P, e)
    mpack[:, :, 2:, :] = (
        np.asarray(mask_neg, dtype=np.float32).reshape(ntiles, P, k, e)
    )

    # pair keys: slot pair (2s, 2s+1) of each element
    pk = ids_all.reshape(b, pairs, 2)
    keys_all = pk[:, :, 0] * (2 * vocab) + pk[:, :, 1]   # [b, pairs] int64

    in_maps = []
    for c in range(ncores):
        lo = c * bsh
        wtab = np.zeros((nsg * TCAP, e), dtype=bf16)
        gi_list = np.empty((nsg, sg_pairs), dtype=np.int16)
        for sg in range(nsg):
            rlo = lo + sg * sg_tiles * P
            kb = keys_all[rlo : rlo + sg_tiles * P]       # [(t p), s]
            # descriptor rank r = dcol*128 + p ; dcol = t_in_sg*pairs + s
            kpos = (
                kb.reshape(sg_tiles, P, pairs)
                .transpose(0, 2, 1)
                .reshape(-1)
            )  # indexed by (t, s, p) = rank order
            uk, inv = np.unique(kpos, return_inverse=True)
            nu = uk.shape[0]
            assert 2 * nu <= TCAP - 1, (nu, TCAP)
            rows = np.empty(2 * nu, dtype=np.int64)
            rows[0::2] = uk // (2 * vocab)
            rows[1::2] = uk % (2 * vocab)
            wtab[sg * TCAP : sg * TCAP + 2 * nu] = W[rows]
            gi_list[sg] = inv.astype(np.int16)
        gim = np.concatenate(
            [
                _wrap16(gi_list[sg][h * (sg_pairs // 2) : (h + 1) * (sg_pairs // 2)],
                        gi_cols // 2)
                for sg in range(nsg)
                for h in range(2)
            ],
            axis=1,
        )
        tlo = lo // P
        mkc = np.ascontiguousarray(
            mpack[tlo : tlo + tiles].transpose(1, 0, 2, 3)
        ).reshape(P, tiles * slots * e)
        in_maps.append(
            {
                "w": wtab.reshape(-1, 2 * e),
                "gi": gim,
                "mk": mkc,
            }
        )
    return in_maps


def kernel(
    ctx_words, target_words, neg_words, V_emb, U_emb, mask_v, mask_u, mask_neg
):
    from concourse.bass_utils import run_bass_kernel_spmd

    b, k = neg_words.shape
    vocab, e = V_emb.shape
    bsh = b // NCORES

    nc = _get_program(bsh, vocab, k, e, SG_TILES, NCORES)
    in_maps = _host_prep(
        ctx_words, target_words, neg_words, V_emb, U_emb,
        mask_v, mask_u, mask_neg, NCORES, SG_TILES,
    )
    res = run_bass_kernel_spmd(nc, in_maps, core_ids=list(range(NCORES)))
    total = np.float64(0.0)
    for c in range(NCORES):
        total += np.float64(
            res.results[c]["out"].astype(np.float64).sum()
        )
    return np.float32(total / b)
